# revision 16
# baseline (speedup 1.0000x reference)
"""Block-Gibbs spin sampler on 8 Trainium2 NeuronCores (Bass/Tile).

Strategy (pure data-parallel over chains, 2048/8 = 256 chains per core):
  - Spins are kept transposed and 0/1-encoded in SBUF: cT[node, chain]
    with c = (x+1)/2, one wide [128, 32*256] fp32 tensor per color,
    resident across all 6 half-sweeps.
  - The padded gather  field = sum_k quadratic[adj_w] * xz[:, adj]  is a
    banded circulant. With x = 2c-1:  field = sum 2J*c - rowsum(J), so
    each output tile is TWO K=128 fp32 matmuls against host-built banded
    lhsT matrices holding 2*quadratic; rowsum(J) folds into thresholds.
  - The Gibbs acceptance  u < sigmoid(-2*beta*field)  is converted on host
    to  field2 < T3,  T3 = -logit(u)/(2*beta) - linear + rowsum(J)  (fp64),
    since u is data-independent (jax.random, key 42, same backend as the
    reference). No on-device transcendentals, no affine: the DVE is_gt
    output (1.0/0.0) IS the next spin state.
  - Per tile on device: 2 matmuls (PE, fp32, PSUM accumulate) ->
    tensor_tensor is_gt (DVE, thr > field -> c') written in place into the
    resident c tensor. Final two half-sweeps DMA c out in 1 MiB chunks;
    the host decodes x = 2c-1.

HBM traffic per core: 24 MiB thresholds + 4 x1-in + 8 out + 8 bands
~ 44 MiB at ~360 GB/s; PE: 384 fp32 matmuls (4 cyc/row) ~ 170 us.
"""

import numpy as np
from contextlib import ExitStack

H = 4096          # nodes per color
NN = 2 * H        # total nodes
MD = 10           # max degree incl. padding
C = 2048          # chains
P = 128           # partitions
T = H // P        # 32 node tiles per color
NCORES = 8
CC = C // NCORES  # 256 chains per core
THB = 8           # tiles per 1 MiB DMA chunk

_PROGRAM_CACHE = {}


# ---------------------------------------------------------------- host side

def _build_bands(adj, adj_w, quadratic, block, src_base):
    """Banded lhsT weights. For out node o = 128*t + r of this block, each
    valid neighbor (adj >= 0) is a row j of source tile u; fold quadratic
    into lhsT[j, r] of the (t, u) matmul. Band property: u is t, t+1 or t-1
    (mod T). Returns (mains[T,128,128], wraps_p, wraps_m, rowsum[H])."""
    A = adj[block]            # [H, MD]
    W = adj_w[block]
    valid = A >= 0
    o = np.broadcast_to(np.arange(H)[:, None], A.shape)
    s = A - src_base
    if not np.all((s[valid] >= 0) & (s[valid] < H)):
        raise AssertionError("neighbor outside opposite color block")
    t, r = np.divmod(o, P)
    u, j = np.divmod(np.where(valid, s, 0), P)
    du = (u - t) % T
    Jv = quadratic[np.where(valid, W, 0)]
    mains = np.zeros((T, P, P), np.float32)
    wraps_p = np.zeros((T, P, P), np.float32)
    wraps_m = np.zeros((T, P, P), np.float32)
    for dest, want in ((mains, 0), (wraps_p, 1), (wraps_m, T - 1)):
        m = valid & (du == want)
        np.add.at(dest, (t[m], j[m], r[m]), Jv[m])
    covered = valid & np.isin(du, [0, 1, T - 1])
    if not np.all(covered == valid):
        raise AssertionError("graph is not banded within +-1 tile")
    rowsum = np.where(valid, Jv.astype(np.float64), 0.0).sum(axis=1)   # [H]
    return mains, wraps_p, wraps_m, rowsum


def _pack_band(band3):
    """[T, 128(j), 128(r)] -> SBUF layout [128(j), T*128 (t-major, r-minor)],
    scaled by 2 for the 0/1 spin encoding."""
    return np.ascontiguousarray(
        (2.0 * band3).astype(np.float32).transpose(1, 0, 2).reshape(P, T * P))


def _to_wide(a):
    """[H, CC] node-major -> [128, T*CC]: partition p, free t*CC+c holds
    node t*128+p, chain c."""
    return np.ascontiguousarray(
        a.reshape(T, P, CC).transpose(1, 0, 2).reshape(P, T * CC))


def _from_wide(a):
    return a.reshape(P, T, CC).transpose(1, 0, 2).reshape(H, CC)


def _host_prepare(inputs):
    x = np.asarray(inputs["x"], np.float32)
    linear = np.asarray(inputs["linear"], np.float32)
    quadratic = np.asarray(inputs["quadratic"], np.float32)
    schedule = np.asarray(inputs["schedule"], np.float32)
    adj = np.asarray(inputs["adj"])
    adj_w = np.asarray(inputs["adj_w"])
    block0 = np.asarray(inputs["block0"])
    block1 = np.asarray(inputs["block1"])
    assert x.shape == (C, NN) and schedule.shape[0] * 2 == 6
    assert np.array_equal(block0, np.arange(H))
    assert np.array_equal(block1, np.arange(H, NN))

    mainsA, wrapsA_p, wrapsA_m, rowsumA = _build_bands(adj, adj_w, quadratic, block0, H)
    mainsB, wrapsB_p, wrapsB_m, rowsumB = _build_bands(adj, adj_w, quadratic, block1, 0)
    # block0 band is [0..+7] (uses t, t+1); block1 band is [-7..0] (t-1, t)
    assert not wrapsA_m.any() and not wrapsB_p.any()

    band_maps = {
        "mainA_in": _pack_band(mainsA),
        "wrapA_in": _pack_band(wrapsA_p),
        "mainB_in": _pack_band(mainsB),
        "wrapB_in": _pack_band(wrapsB_m),
    }

    # thresholds: c'=1 iff sum(2J c) < T3,  T3 = -logit(u)/(2b) - lin + rowsum
    # u must be bit-identical to the reference's jax.random stream, which is
    # backend-dependent (this env pins jax_default_prng_impl=rbg): generate it
    # with the same calls on the same default backend as the reference.
    import os
    u_cache_path = os.environ.get("SPIN_U_CACHE", "")
    u_cached = None
    if u_cache_path and os.path.exists(u_cache_path):
        u_cached = np.load(u_cache_path)
    u_save = {}
    if u_cached is None:
        import jax
        key = jax.random.key(42)
    thr = np.empty((6, C, H), np.float32)
    for s in range(6):
        tstep, b = divmod(s, 2)
        beta = np.float64(schedule[tstep])
        if u_cached is not None:
            u = u_cached[f"u{s}"]
        else:
            sub = jax.random.fold_in(key, s)
            u = np.asarray(jax.random.uniform(sub, (C, H), dtype=np.float32))
            u_save[f"u{s}"] = u
        u64 = u.astype(np.float64)
        with np.errstate(divide="ignore"):
            logit = np.log(u64) - np.log1p(-u64)
        lin = linear[np.asarray(block0 if b == 0 else block1)].astype(np.float64)
        rs = rowsumA if b == 0 else rowsumB
        thr[s] = ((-logit / (2.0 * beta)) - lin[None, :] + rs[None, :]).astype(np.float32)
    if u_cache_path and u_save and not os.path.exists(u_cache_path):
        try:
            np.savez(u_cache_path, **u_save)
        except OSError:
            pass

    in_maps = []
    for core in range(NCORES):
        c0 = core * CC
        c1T = np.ascontiguousarray((x[c0:c0 + CC, H:].T + 1.0) * 0.5)   # [H, CC]
        thrT = thr[:, c0:c0 + CC, :].transpose(0, 2, 1)                  # [6, H, CC]
        in_maps.append({
            "x1_in": _to_wide(c1T),
            "thr_in": np.stack([_to_wide(thrT[s]) for s in range(6)]),
            **band_maps,
        })
    return in_maps


# -------------------------------------------------------------- device side

def _build_program():
    import concourse.bacc as bacc
    import concourse.mybir as mybir
    import concourse.tile as tile
    from concourse.bass import ts
    from concourse.alu_op_type import AluOpType

    f32 = mybir.dt.float32
    nc = bacc.Bacc("TRN2", target_bir_lowering=False, debug=False,
                   enable_asserts=False, num_devices=NCORES)

    x1_in = nc.dram_tensor("x1_in", [P, T * CC], f32, kind="ExternalInput")
    thr_in = nc.dram_tensor("thr_in", [6, P, T * CC], f32, kind="ExternalInput")
    mainA_in = nc.dram_tensor("mainA_in", [P, T * P], f32, kind="ExternalInput")
    wrapA_in = nc.dram_tensor("wrapA_in", [P, T * P], f32, kind="ExternalInput")
    mainB_in = nc.dram_tensor("mainB_in", [P, T * P], f32, kind="ExternalInput")
    wrapB_in = nc.dram_tensor("wrapB_in", [P, T * P], f32, kind="ExternalInput")
    x0_out = nc.dram_tensor("x0_out", [P, T * CC], f32, kind="ExternalOutput")
    x1_out = nc.dram_tensor("x1_out", [P, T * CC], f32, kind="ExternalOutput")

    with tile.TileContext(nc) as tc, ExitStack() as ctx:
        xp = ctx.enter_context(tc.tile_pool(name="xp", bufs=1))
        bandp = ctx.enter_context(tc.tile_pool(name="bandp", bufs=1))
        thp = ctx.enter_context(tc.tile_pool(name="thp", bufs=5))
        psp = ctx.enter_context(tc.tile_pool(name="psp", bufs=6, space="PSUM"))

        xw0 = xp.tile([P, T * CC], f32, name="xw0", tag="xw0")
        xw1 = xp.tile([P, T * CC], f32, name="xw1", tag="xw1")
        mainA = bandp.tile([P, T * P], f32, name="mainA", tag="mainA")
        wrapA = bandp.tile([P, T * P], f32, name="wrapA", tag="wrapA")
        mainB = bandp.tile([P, T * P], f32, name="mainB", tag="mainB")
        wrapB = bandp.tile([P, T * P], f32, name="wrapB", tag="wrapB")

        nc.sync.dma_start(out=mainA, in_=mainA_in[:])
        nc.sync.dma_start(out=wrapA, in_=wrapA_in[:])
        nc.sync.dma_start(out=mainB, in_=mainB_in[:])
        nc.sync.dma_start(out=wrapB, in_=wrapB_in[:])
        for q in range(T // THB):
            nc.sync.dma_start(out=xw1[:, ts(q, THB * CC)],
                              in_=x1_in[:, ts(q, THB * CC)])

        # cyclic start offsets keep the cross-sweep wavefront 1-2 tiles deep
        start_off = [0, 1, 1, 2, 2, 3]
        for s in range(6):
            b = s % 2
            src = xw1 if b == 0 else xw0
            dst = xw0 if b == 0 else xw1
            main_band = mainA if b == 0 else mainB
            wrap_band = wrapA if b == 0 else wrapB
            dst_dram = x0_out if s == 4 else (x1_out if s == 5 else None)
            th_wide = {}
            chunk_left = [THB] * (T // THB)
            for idx in range(T):
                t = (start_off[s] + idx) % T
                q = t // THB                     # 1 MiB threshold chunk
                if q not in th_wide:
                    thw = thp.tile([P, THB * CC], f32, name=f"th_{s}_{q}", tag="th")
                    # ACT HWDGE ring: independent FIFO from the sync-ring DMAs
                    nc.scalar.dma_start(out=thw, in_=thr_in[s, :, q * THB * CC:(q + 1) * THB * CC])
                    th_wide[q] = thw
                ps = psp.tile([P, CC], f32, name=f"ps_{s}_{t}", tag="ps")
                if b == 0:   # band [0..+7]: src tiles t, t+1 (K ascending)
                    nc.tensor.matmul(ps, lhsT=main_band[:, ts(t, P)],
                                     rhs=src[:, ts(t, CC)], start=True, stop=False)
                    nc.tensor.matmul(ps, lhsT=wrap_band[:, ts(t, P)],
                                     rhs=src[:, ts((t + 1) % T, CC)], start=False, stop=True)
                else:        # band [-7..0]: src tiles t-1, t (K ascending)
                    nc.tensor.matmul(ps, lhsT=wrap_band[:, ts(t, P)],
                                     rhs=src[:, ts((t - 1) % T, CC)], start=True, stop=False)
                    nc.tensor.matmul(ps, lhsT=main_band[:, ts(t, P)],
                                     rhs=src[:, ts(t, CC)], start=False, stop=True)
                # thr > field -> 1.0/0.0: directly the 0/1-encoded next state
                nc.vector.tensor_tensor(out=dst[:, ts(t, CC)],
                                        in0=th_wide[q][:, ts(t % THB, CC)],
                                        in1=ps, op=AluOpType.is_gt)
                if dst_dram is not None:
                    chunk_left[q] -= 1
                    if chunk_left[q] == 0:
                        nc.sync.dma_start(out=dst_dram[:, ts(q, THB * CC)],
                                          in_=dst[:, ts(q, THB * CC)])

    nc.compile()
    return nc


def get_program():
    if "nc" not in _PROGRAM_CACHE:
        _PROGRAM_CACHE["nc"] = _build_program()
    return _PROGRAM_CACHE["nc"]


# ------------------------------------------------------------------- driver

def kernel(**inputs) -> np.ndarray:
    in_maps = _host_prepare(inputs)
    nc = get_program()
    from concourse.bass_utils import run_bass_kernel_spmd
    res = run_bass_kernel_spmd(nc, in_maps, core_ids=list(range(NCORES)))
    out = np.empty((C, NN), np.float32)
    for core in range(NCORES):
        c0 = core * CC
        r = res.results[core]
        out[c0:c0 + CC, :H] = (2.0 * _from_wide(r["x0_out"]) - 1.0).T
        out[c0:c0 + CC, H:] = (2.0 * _from_wide(r["x1_out"]) - 1.0).T
    return out


# revision 17
# speedup vs baseline: 1.6123x; 1.6123x over previous
"""Block-Gibbs spin sampler on 8 Trainium2 NeuronCores (Bass/Tile).

Strategy (pure data-parallel over chains, 2048/8 = 256 chains per core):
  - Spins are kept transposed and 0/1-encoded in SBUF: cT[node, chain]
    with c = (x+1)/2, one wide [128, 32*256] fp32 tensor per color,
    resident across all 6 half-sweeps.
  - The padded gather  field = sum_k quadratic[adj_w] * xz[:, adj]  is a
    banded circulant. With x = 2c-1:  field = sum 2J*c - rowsum(J), so
    each output tile is TWO K=128 fp32 matmuls against host-built banded
    lhsT matrices holding 2*quadratic; rowsum(J) folds into thresholds.
  - The Gibbs acceptance  u < sigmoid(-2*beta*field)  is converted on host
    to  field2 < T3,  T3 = -logit(u)/(2*beta) - linear + rowsum(J)  (fp64),
    since u is data-independent (jax.random, key 42, same backend as the
    reference). No on-device transcendentals, no affine: the DVE is_gt
    output (1.0/0.0) IS the next spin state.
  - Per tile on device: 2 matmuls (PE, fp32, PSUM accumulate) ->
    tensor_tensor is_gt (DVE, thr > field -> c') written in place into the
    resident c tensor. Final two half-sweeps DMA c out in 1 MiB chunks;
    the host decodes x = 2c-1.

HBM traffic per core: 24 MiB thresholds + 4 x1-in + 8 out + 8 bands
~ 44 MiB at ~360 GB/s; PE: 384 fp32 matmuls (4 cyc/row) ~ 170 us.
"""

import numpy as np
from contextlib import ExitStack

H = 4096          # nodes per color
NN = 2 * H        # total nodes
MD = 10           # max degree incl. padding
C = 2048          # chains
P = 128           # partitions
T = H // P        # 32 node tiles per color
NCORES = 8
CC = C // NCORES  # 256 chains per core
THB = 8           # tiles per 1 MiB DMA chunk

_PROGRAM_CACHE = {}


# ---------------------------------------------------------------- host side

def _build_bands(adj, adj_w, quadratic, block, src_base):
    """Banded lhsT weights. For out node o = 128*t + r of this block, each
    valid neighbor (adj >= 0) is a row j of source tile u; fold quadratic
    into lhsT[j, r] of the (t, u) matmul. Band property: u is t, t+1 or t-1
    (mod T). Returns (mains[T,128,128], wraps_p, wraps_m, rowsum[H])."""
    A = adj[block]            # [H, MD]
    W = adj_w[block]
    valid = A >= 0
    o = np.broadcast_to(np.arange(H)[:, None], A.shape)
    s = A - src_base
    if not np.all((s[valid] >= 0) & (s[valid] < H)):
        raise AssertionError("neighbor outside opposite color block")
    t, r = np.divmod(o, P)
    u, j = np.divmod(np.where(valid, s, 0), P)
    du = (u - t) % T
    Jv = quadratic[np.where(valid, W, 0)]
    mains = np.zeros((T, P, P), np.float32)
    wraps_p = np.zeros((T, P, P), np.float32)
    wraps_m = np.zeros((T, P, P), np.float32)
    for dest, want in ((mains, 0), (wraps_p, 1), (wraps_m, T - 1)):
        m = valid & (du == want)
        np.add.at(dest, (t[m], j[m], r[m]), Jv[m])
    covered = valid & np.isin(du, [0, 1, T - 1])
    if not np.all(covered == valid):
        raise AssertionError("graph is not banded within +-1 tile")
    rowsum = np.where(valid, Jv.astype(np.float64), 0.0).sum(axis=1)   # [H]
    return mains, wraps_p, wraps_m, rowsum


def _pack_band(band3):
    """[T, 128(j), 128(r)] -> fp16 (hi, lo) SBUF layouts [128(j), T*128],
    scaled by 2 for the 0/1 spin encoding. hi+lo represents 2J to ~2^-22
    relative (PE honors fp16 denormal weights, verified on HW)."""
    w = (2.0 * band3).astype(np.float32).transpose(1, 0, 2).reshape(P, T * P)
    hi = w.astype(np.float16)
    lo = (w - hi.astype(np.float32)).astype(np.float16)
    return np.ascontiguousarray(hi), np.ascontiguousarray(lo)


def _to_wide(a):
    """[H, CC] node-major -> [128, T*CC]: partition p, free t*CC+c holds
    node t*128+p, chain c."""
    return np.ascontiguousarray(
        a.reshape(T, P, CC).transpose(1, 0, 2).reshape(P, T * CC))


def _from_wide(a):
    return a.reshape(P, T, CC).transpose(1, 0, 2).reshape(H, CC)


def _host_prepare(inputs):
    x = np.asarray(inputs["x"], np.float32)
    linear = np.asarray(inputs["linear"], np.float32)
    quadratic = np.asarray(inputs["quadratic"], np.float32)
    schedule = np.asarray(inputs["schedule"], np.float32)
    adj = np.asarray(inputs["adj"])
    adj_w = np.asarray(inputs["adj_w"])
    block0 = np.asarray(inputs["block0"])
    block1 = np.asarray(inputs["block1"])
    assert x.shape == (C, NN) and schedule.shape[0] * 2 == 6
    assert np.array_equal(block0, np.arange(H))
    assert np.array_equal(block1, np.arange(H, NN))

    mainsA, wrapsA_p, wrapsA_m, rowsumA = _build_bands(adj, adj_w, quadratic, block0, H)
    mainsB, wrapsB_p, wrapsB_m, rowsumB = _build_bands(adj, adj_w, quadratic, block1, 0)
    # block0 band is [0..+7] (uses t, t+1); block1 band is [-7..0] (t-1, t)
    assert not wrapsA_m.any() and not wrapsB_p.any()

    band_maps = {}
    for name, band in (("mainA", mainsA), ("wrapA", wrapsA_p),
                       ("mainB", mainsB), ("wrapB", wrapsB_m)):
        band_maps[name + "_hi_in"], band_maps[name + "_lo_in"] = _pack_band(band)

    # thresholds: c'=1 iff sum(2J c) < T3,  T3 = -logit(u)/(2b) - lin + rowsum
    # u must be bit-identical to the reference's jax.random stream, which is
    # backend-dependent (this env pins jax_default_prng_impl=rbg): generate it
    # with the same calls on the same default backend as the reference.
    import os
    u_cache_path = os.environ.get("SPIN_U_CACHE", "")
    u_cached = None
    if u_cache_path and os.path.exists(u_cache_path):
        u_cached = np.load(u_cache_path)
    u_save = {}
    if u_cached is None:
        import jax
        key = jax.random.key(42)
    thr = np.empty((6, C, H), np.float32)
    for s in range(6):
        tstep, b = divmod(s, 2)
        beta = np.float64(schedule[tstep])
        if u_cached is not None:
            u = u_cached[f"u{s}"]
        else:
            sub = jax.random.fold_in(key, s)
            u = np.asarray(jax.random.uniform(sub, (C, H), dtype=np.float32))
            u_save[f"u{s}"] = u
        u64 = u.astype(np.float64)
        with np.errstate(divide="ignore"):
            logit = np.log(u64) - np.log1p(-u64)
        lin = linear[np.asarray(block0 if b == 0 else block1)].astype(np.float64)
        rs = rowsumA if b == 0 else rowsumB
        thr[s] = ((-logit / (2.0 * beta)) - lin[None, :] + rs[None, :]).astype(np.float32)
    if u_cache_path and u_save and not os.path.exists(u_cache_path):
        try:
            np.savez(u_cache_path, **u_save)
        except OSError:
            pass

    in_maps = []
    for core in range(NCORES):
        c0 = core * CC
        c1T = np.ascontiguousarray((x[c0:c0 + CC, H:].T + 1.0) * 0.5).astype(np.float16)
        thrT = thr[:, c0:c0 + CC, :].transpose(0, 2, 1)                  # [6, H, CC]
        in_maps.append({
            "x1_in": _to_wide(c1T),
            "thr_in": np.stack([_to_wide(thrT[s]) for s in range(6)]),
            **band_maps,
        })
    return in_maps


# -------------------------------------------------------------- device side

def _build_program():
    import concourse.bacc as bacc
    import concourse.mybir as mybir
    import concourse.tile as tile
    from concourse.bass import ts
    from concourse.alu_op_type import AluOpType

    f32 = mybir.dt.float32
    nc = bacc.Bacc("TRN2", target_bir_lowering=False, debug=False,
                   enable_asserts=False, num_devices=NCORES)

    f16 = mybir.dt.float16
    x1_in = nc.dram_tensor("x1_in", [P, T * CC], f16, kind="ExternalInput")
    thr_in = nc.dram_tensor("thr_in", [6, P, T * CC], f32, kind="ExternalInput")
    band_ins = {n: nc.dram_tensor(n + "_in", [P, T * P], f16, kind="ExternalInput")
                for n in ("mainA_hi", "mainA_lo", "wrapA_hi", "wrapA_lo",
                          "mainB_hi", "mainB_lo", "wrapB_hi", "wrapB_lo")}
    x0_out = nc.dram_tensor("x0_out", [P, T * CC], f16, kind="ExternalOutput")
    x1_out = nc.dram_tensor("x1_out", [P, T * CC], f16, kind="ExternalOutput")

    with tile.TileContext(nc) as tc, ExitStack() as ctx:
        xp = ctx.enter_context(tc.tile_pool(name="xp", bufs=1))
        bandp = ctx.enter_context(tc.tile_pool(name="bandp", bufs=1))
        thp = ctx.enter_context(tc.tile_pool(name="thp", bufs=5))
        psp = ctx.enter_context(tc.tile_pool(name="psp", bufs=6, space="PSUM"))

        xw0 = xp.tile([P, T * CC], f16, name="xw0", tag="xw0")
        xw1 = xp.tile([P, T * CC], f16, name="xw1", tag="xw1")
        bands = {}
        for n, dram in band_ins.items():
            bands[n] = bandp.tile([P, T * P], f16, name=n, tag=n)
            nc.sync.dma_start(out=bands[n], in_=dram[:])
        for q in range(T // THB):
            nc.sync.dma_start(out=xw1[:, ts(q, THB * CC)],
                              in_=x1_in[:, ts(q, THB * CC)])

        # cyclic start offsets keep the cross-sweep wavefront 1-2 tiles deep
        start_off = [0, 1, 1, 2, 2, 3]
        for s in range(6):
            b = s % 2
            src = xw1 if b == 0 else xw0
            dst = xw0 if b == 0 else xw1
            mh = bands["mainA_hi" if b == 0 else "mainB_hi"]
            ml = bands["mainA_lo" if b == 0 else "mainB_lo"]
            wh = bands["wrapA_hi" if b == 0 else "wrapB_hi"]
            wl = bands["wrapA_lo" if b == 0 else "wrapB_lo"]
            dst_dram = x0_out if s == 4 else (x1_out if s == 5 else None)
            th_wide = {}
            chunk_left = [THB] * (T // THB)
            for idx in range(T):
                t = (start_off[s] + idx) % T
                q = t // THB                     # 1 MiB threshold chunk
                if q not in th_wide:
                    thw = thp.tile([P, THB * CC], f32, name=f"th_{s}_{q}", tag="th")
                    # ACT HWDGE ring: independent FIFO from the sync-ring DMAs
                    nc.scalar.dma_start(out=thw, in_=thr_in[s, :, q * THB * CC:(q + 1) * THB * CC])
                    th_wide[q] = thw
                ps = psp.tile([P, CC], f32, name=f"ps_{s}_{t}", tag="ps")
                if b == 0:   # band [0..+7]: src tiles t, t+1 (K ascending)
                    nc.tensor.matmul(ps, lhsT=mh[:, ts(t, P)],
                                     rhs=src[:, ts(t, CC)], start=True, stop=False)
                    nc.tensor.matmul(ps, lhsT=ml[:, ts(t, P)],
                                     rhs=src[:, ts(t, CC)], start=False, stop=False)
                    nc.tensor.matmul(ps, lhsT=wh[:, ts(t, P)],
                                     rhs=src[:, ts((t + 1) % T, CC)], start=False, stop=False)
                    nc.tensor.matmul(ps, lhsT=wl[:, ts(t, P)],
                                     rhs=src[:, ts((t + 1) % T, CC)], start=False, stop=True)
                else:        # band [-7..0]: src tiles t-1, t (K ascending)
                    nc.tensor.matmul(ps, lhsT=wh[:, ts(t, P)],
                                     rhs=src[:, ts((t - 1) % T, CC)], start=True, stop=False)
                    nc.tensor.matmul(ps, lhsT=wl[:, ts(t, P)],
                                     rhs=src[:, ts((t - 1) % T, CC)], start=False, stop=False)
                    nc.tensor.matmul(ps, lhsT=mh[:, ts(t, P)],
                                     rhs=src[:, ts(t, CC)], start=False, stop=False)
                    nc.tensor.matmul(ps, lhsT=ml[:, ts(t, P)],
                                     rhs=src[:, ts(t, CC)], start=False, stop=True)
                # thr > field -> 1.0/0.0: directly the 0/1-encoded next state
                nc.vector.tensor_tensor(out=dst[:, ts(t, CC)],
                                        in0=th_wide[q][:, ts(t % THB, CC)],
                                        in1=ps, op=AluOpType.is_gt)
                if dst_dram is not None:
                    chunk_left[q] -= 1
                    if chunk_left[q] == 0:
                        nc.sync.dma_start(out=dst_dram[:, ts(q, THB * CC)],
                                          in_=dst[:, ts(q, THB * CC)])

    nc.compile()
    return nc


def get_program():
    if "nc" not in _PROGRAM_CACHE:
        _PROGRAM_CACHE["nc"] = _build_program()
    return _PROGRAM_CACHE["nc"]


# ------------------------------------------------------------------- driver

def kernel(**inputs) -> np.ndarray:
    in_maps = _host_prepare(inputs)
    nc = get_program()
    from concourse.bass_utils import run_bass_kernel_spmd
    res = run_bass_kernel_spmd(nc, in_maps, core_ids=list(range(NCORES)))
    out = np.empty((C, NN), np.float32)
    for core in range(NCORES):
        c0 = core * CC
        r = res.results[core]
        out[c0:c0 + CC, :H] = (2.0 * _from_wide(r["x0_out"]) - 1.0).T
        out[c0:c0 + CC, H:] = (2.0 * _from_wide(r["x1_out"]) - 1.0).T
    return out
